# revision 34
# baseline (speedup 1.0000x reference)
"""Trainium2 Bass kernel for nn_AttentionBlockConv_55121610276882.

Strategy (8 cores, full I/O):
  core c -> (batch b = c//4, query-strip r = c%4).
  Each core: full ConvBlock for its batch image, k/v convs + 2x2 maxpool for
  the full key set, then flash-style attention for its 4736-query window
  (37 rows = 32-row strip + conv6 halo).  The joint softmax uses exp(s)
  directly (M=0 is safe: |s| < 1); the global normalizer Z is finished on
  the host.  The tail is decomposed linearly:
      out = conv6(inp) + (sigma/Z) * conv6(W_last * ap_unnorm)
  so each core returns A = conv6(inp_strip), B = conv6(lc_strip), z-partials.

  Convs run as K-packed matmuls WITHOUT tile_position (which faults on this
  runtime): the input plane is stored in Q dx-shifted partition groups
  (group q holds the plane shifted left by q columns), so one matmul
  contracts over (dx-group x channel) = K<=120 and the dy taps become
  free-dim row offsets chunked over accumulating matmuls.  The group
  replication is free: the producing conv's lhsT duplicates its output
  columns Q times, and the PSUM->SBUF eviction writes each group at its
  shifted column range (same partitions, so DVE/ACT can do it).

  Per-core divergence (which 37-row query window) is data-driven: a
  per-core one-hot selection matrix picks 1 of 4 statically-gathered strips
  via a matmul, so all 8 cores run an identical program.
"""
import sys

sys.path.insert(0, '/opt/trn_rl_repo')

import numpy as np
import ml_dtypes
import concourse.bass as bass
import concourse.mybir as mybir
import concourse.tile as tile
from concourse.bass_utils import run_bass_kernel_spmd
from concourse.masks import make_identity

F32 = mybir.dt.float32
BF = mybir.dt.bfloat16
AF = mybir.ActivationFunctionType
ALU = mybir.AluOpType

EPS = 1e-3
PI = np.float32(np.pi)

_NC_CACHE = {}


def split_multiwaits(nc, max_waits=1):
    """walrus here rejects >1 sync-wait per instruction (tail Drain).
    Hoist extra waits onto dedicated NOPs."""
    n_new = 0
    for fn in nc.m.functions:
        for bb in fn.blocks:
            insts = bb.instructions
            i = 0
            while i < len(insts):
                inst = insts[i]
                si = inst.sync_info
                if si is not None and len(si.on_wait) > max_waits:
                    waits = list(si.on_wait)
                    keep = waits[-max_waits:]
                    extra = waits[:-max_waits]
                    nops = []
                    for w in extra:
                        nops.append(mybir.InstNoOp(
                            name=f"{inst.name}-ws{n_new}",
                            engine=inst.engine,
                            ins=[], outs=[],
                            sync_info=mybir.SyncInfo(on_wait=[w], on_update=[]),
                        ))
                        n_new += 1
                    inst.sync_info = mybir.SyncInfo(on_wait=keep,
                                                    on_update=list(si.on_update))
                    for j, nop in enumerate(nops):
                        insts.insert(i + j, nop)
                    i += len(nops)
                i += 1
    return n_new


# chunk lists: (dy, dx_extra, kparts)
CH2 = [(d, 0, 100) for d in range(5)]
CH3 = [(d, 0, 120) for d in range(5)] + [(d, 4, 30) for d in range(5)]
CH4 = [(d, 0, 120) for d in range(5)] + [(d, 3, 80) for d in range(5)]
CHAP = [(d, 0, 100) for d in range(3)] + [(d, 2, 50) for d in range(3)]


def build_nc(for_hw=True, phases=6):
    nc = bass.Bass("TRN2")

    io = {}

    def dinp(name, shape, dt=F32):
        io[name] = nc.dram_tensor(name, shape, dt, kind="ExternalInput")

    dinp("P", [128, 16896], BF)
    dinp("lhs1", [128, 40], BF)
    dinp("b1", [20, 1]); dinp("s2", [20, 1]); dinp("t2", [20, 1])
    dinp("W2sb", [128, len(CH2), 30], BF)
    dinp("b2", [30, 1]); dinp("s3", [30, 1]); dinp("t3", [30, 1])
    dinp("W3sb", [128, len(CH3), 40], BF)
    dinp("b3", [40, 1]); dinp("s4", [40, 1]); dinp("t4", [40, 1])
    dinp("W4sb", [128, len(CH4), 50], BF)
    dinp("b4", [50, 1])
    dinp("Wap", [128, len(CHAP), 8], BF)
    dinp("Wk", [8, 8], BF); dinp("Wv", [8, 8], BF); dinp("Wq", [8, 8], BF)
    dinp("Wl", [8, 8], BF)
    dinp("W6sb", [48, 6, 4], BF)
    dinp("sel4", [32, 8], BF)
    dinp("mask", [8, 37, 128])

    io["A"] = nc.dram_tensor("A", [4, 4736], F32, kind="ExternalOutput")
    io["Bo"] = nc.dram_tensor("Bo", [4, 4736], F32, kind="ExternalOutput")
    io["z"] = nc.dram_tensor("z", [1, 4736], F32, kind="ExternalOutput")
    io["stage"] = nc.dram_tensor("stage", [8, 133, 128], BF, kind="Internal")

    with tile.TileContext(nc) as tc:
        _emit(nc, tc, io, phases)

    if for_hw:
        split_multiwaits(nc)
    return nc


_DMA_ENGS = None


def _dma_rr(nc):
    """Round-robin DMA issue across engine sequencers."""
    global _DMA_ENGS
    if _DMA_ENGS is None or _DMA_ENGS[0] is not nc:
        _DMA_ENGS = (nc, [nc.sync, nc.scalar, nc.gpsimd], [0])
    engs, ctr = _DMA_ENGS[1], _DMA_ENGS[2]
    e = engs[ctr[0] % len(engs)]
    ctr[0] += 1
    return e


def _i2c_flush(nc, plane, batch, cch, Q, t0, nrows, padl):
    """DMA the batch scratch [cch, nrows, 128] into Q shifted plane groups;
    batch rows correspond to plane rows padl+t0*4 .. +nrows."""
    for q in range(Q):
        lo = padl - q
        srcl = 0
        if lo < 0:
            srcl = -lo
            lo = 0
        width = 128 - srcl
        dsl = plane[cch * q:cch * (q + 1),
                    padl + 4 * t0:padl + 4 * t0 + nrows, lo:lo + width]
        ssl = batch[0:cch, 0:nrows, srcl:128]
        _dma_rr(nc).dma_start(out=dsl, in_=ssl)


def _zero_plane_borders(nc, plane, H, W, prl, prh, pcl, Q, cch):
    """Row borders, left cols, and per-group right/unwritten column ranges."""
    nc.gpsimd.memset(plane[:, 0:prl, :], 0.0)
    nc.gpsimd.memset(plane[:, H - prh:H, :], 0.0)
    if pcl > 0:
        nc.gpsimd.memset(plane[:, :, 0:pcl], 0.0)
    st = pcl + 128 - (Q - 1)
    if st < W:
        nc.gpsimd.memset(plane[:, :, st:W], 0.0)


def _emit(nc, tc, T, phases=6):
    from contextlib import ExitStack
    ctx = ExitStack()
    with ctx:
        pool_w = ctx.enter_context(tc.tile_pool(name="weights", bufs=1))
        pool_long = ctx.enter_context(tc.tile_pool(name="long", bufs=1))
        pool_ev = ctx.enter_context(tc.tile_pool(name="evict", bufs=2))

        def wtile(name, shape, dt=F32):
            t = pool_w.tile(shape, dt, name=f"w_{name}")
            nc.sync.dma_start(out=t, in_=T[name][tuple(slice(None) for _ in shape)])
            return t

        def wconv(name, shape):
            t = pool_w.tile(shape, BF, tag="wconv", name=f"w_{name}")
            nc.sync.dma_start(out=t, in_=T[name][tuple(slice(None) for _ in shape)])
            return t

        lhs1 = wconv("lhs1", [128, 40])
        b1 = wtile("b1", [20, 1]); s2 = wtile("s2", [20, 1]); t2 = wtile("t2", [20, 1])
        W2 = wconv("W2sb", [128, len(CH2), 30])
        b2 = wtile("b2", [30, 1]); s3 = wtile("s3", [30, 1]); t3 = wtile("t3", [30, 1])
        W3 = wconv("W3sb", [128, len(CH3), 40])
        b3 = wtile("b3", [40, 1]); s4 = wtile("s4", [40, 1]); t4 = wtile("t4", [40, 1])
        W4 = wconv("W4sb", [128, len(CH4), 50])
        b4 = wtile("b4", [50, 1])
        Wap = wtile("Wap", [128, len(CHAP), 8], BF)
        Wk = wtile("Wk", [8, 8], BF); Wv = wtile("Wv", [8, 8], BF); Wq = wtile("Wq", [8, 8], BF)
        Wl = wtile("Wl", [8, 8], BF)
        W6 = wtile("W6sb", [48, 6, 4], BF)
        sel4 = wtile("sel4", [32, 8], BF)
        ident8 = pool_w.tile([8, 8], BF, name="ident8")
        make_identity(nc, ident8)
        zrow = pool_w.tile([8, 128], BF, name="zrow")
        nc.vector.memset(zrow, 0.0)

        # ---- conv phase ----
        with tc.tile_pool(name="planes", bufs=1) as pool_pl, \
             tc.tile_pool(name="psc", bufs=6, space="PSUM") as psc:
            P_sb = pool_pl.tile([128, 16896], BF, tag="T1", name="P_sb")
            for i in range(4):
                nc.sync.dma_start(out=P_sb[:, i * 4224:(i + 1) * 4224],
                                  in_=T["P"][:, i * 4224:(i + 1) * 4224])
            h1p = pool_pl.tile([100, 132, 132], BF, tag="T2", name="h1p")
            _zero_plane_borders(nc, h1p, 132, 132, 2, 2, 2, 5, 20)

            batch = None
            for t in range(32):
                if phases < 1:
                    break
                ps = psc.tile([20, 4, 128], F32, tag="cps", name="c1ps")
                nc.tensor.matmul(ps[:, :, :], lhs1[:, 0:20],
                                 P_sb[:, 512 * t:512 * t + 512],
                                 start=True, stop=False)
                nc.tensor.matmul(ps[:, :, :], lhs1[:, 20:40],
                                 P_sb[:, 512 * t + 512:512 * t + 1024],
                                 start=False, stop=True)
                sc = pool_ev.tile([20, 4, 128], F32, tag="ev", name="c1sc")
                nc.vector.tensor_scalar(out=sc, in0=ps, scalar1=b1[:, 0:1],
                                        scalar2=0.0, op0=ALU.add, op1=ALU.max)
                if t % 4 == 0:
                    batch = pool_ev.tile([20, 16, 128], BF, tag="bat", name="c1b")
                nc.gpsimd.tensor_scalar(out=batch[:, 4 * (t % 4):4 * (t % 4) + 4, :],
                                        in0=sc, scalar1=s2[:, 0:1], scalar2=t2[:, 0:1],
                                        op0=ALU.mult, op1=ALU.add)
                if t % 4 == 3:
                    _i2c_flush(nc, h1p, batch, 20, 5, t - 3, 16, 2)

            if phases == 1:
                nc.sync.dma_start(out=T["A"][:, :], in_=h1p[0:4, 2:39, 2:130])
                return

            def tap_conv(src, Wsb, chunks, cout, bias, affine, dstplane, Q, padl):
                batch = None
                for t in range(32):
                    ps = psc.tile([cout, 4, 128], F32, tag="cps",
                                  name=f"cps{cout}")
                    for c, (dy, dxe, kp) in enumerate(chunks):
                        nc.tensor.matmul(ps[:, :, :], Wsb[0:kp, c, :],
                                         src[0:kp, 4 * t + dy:4 * t + dy + 4,
                                             dxe:dxe + 128],
                                         start=(c == 0), stop=(c == len(chunks) - 1))
                    if t % 4 == 0:
                        batch = pool_ev.tile([cout, 16, 128], BF, tag="bat",
                                             name=f"b{cout}")
                    bsl = batch[:, 4 * (t % 4):4 * (t % 4) + 4, :]
                    if affine is not None:
                        sc = pool_ev.tile([cout, 4, 128], F32, tag="ev",
                                          name=f"sc{cout}")
                        nc.vector.tensor_scalar(out=sc, in0=ps, scalar1=bias[:, 0:1],
                                                scalar2=0.0, op0=ALU.add, op1=ALU.max)
                        nc.gpsimd.tensor_scalar(out=bsl, in0=sc,
                                                scalar1=affine[0][:, 0:1],
                                                scalar2=affine[1][:, 0:1],
                                                op0=ALU.mult, op1=ALU.add)
                    else:
                        nc.vector.tensor_scalar(out=bsl, in0=ps, scalar1=bias[:, 0:1],
                                                scalar2=0.0, op0=ALU.add, op1=ALU.max)
                    if t % 4 == 3:
                        _i2c_flush(nc, dstplane, batch, cout, Q, t - 3, 16, padl)

            h2p = pool_pl.tile([120, 132, 132], BF, tag="T1", name="h2p")
            _zero_plane_borders(nc, h2p, 132, 132, 2, 2, 2, 4, 30)
            tap_conv(h1p, W2, CH2, 30, b2, (s3, t3), h2p, 4, 2)

            h3p = pool_pl.tile([120, 132, 132], BF, tag="T2", name="h3p")
            _zero_plane_borders(nc, h3p, 132, 132, 2, 2, 2, 3, 40)
            tap_conv(h2p, W3, CH3, 40, b3, (s4, t4), h3p, 3, 2)

            h4p = pool_pl.tile([100, 130, 130], BF, tag="T1", name="h4p")
            _zero_plane_borders(nc, h4p, 130, 130, 1, 1, 1, 2, 50)
            tap_conv(h3p, W4, CH4, 50, b4, None, h4p, 2, 1)

            # amp/phase -> inp_sb [8, 16384] (amp relu, phase plain, pi folded)
            inp_sb = pool_pl.tile([8, 16384], BF, tag="T2", name="inp_sb")
            for t in range(32):
                ps = psc.tile([8, 4, 128], F32, tag="cps", name="apps")
                for c, (dy, dxe, kp) in enumerate(CHAP):
                    nc.tensor.matmul(ps[:, :, :], Wap[0:kp, c, :],
                                     h4p[0:kp, 4 * t + dy:4 * t + dy + 4, dxe:dxe + 128],
                                     start=(c == 0), stop=(c == len(CHAP) - 1))
                nc.vector.tensor_copy(out=inp_sb[0:8, 512 * t:512 * t + 512], in_=ps[0:8])
                nc.scalar.activation(out=inp_sb[0:4, 512 * t:512 * t + 512],
                                     in_=ps[0:4], func=AF.Relu, bias=0.0)

            if phases == 2:
                nc.sync.dma_start(out=T["A"][:, :], in_=inp_sb[0:4, 0:4736])
                return

            # k and v 1x1 convs into bf16 planes, then batched 2x2 maxpool
            kpl = pool_pl.tile([8, 128, 128], BF, tag="kvk", name="kpl")
            vpl = pool_pl.tile([8, 128, 128], BF, tag="kvv", name="vpl")
            for t in range(32):
                for ei, (Wt, dst) in enumerate(((Wk, kpl), (Wv, vpl))):
                    psk = psc.tile([8, 4, 128], F32, tag="cps", name="kvps")
                    nc.tensor.matmul(psk[:, :, :], Wt[:, :],
                                     inp_sb[:, 512 * t:512 * t + 512],
                                     start=True, stop=True)
                    if ei == 0:
                        nc.vector.tensor_copy(out=dst[:, 4 * t:4 * t + 4, :], in_=psk)
                    else:
                        nc.scalar.copy(out=dst[:, 4 * t:4 * t + 4, :], in_=psk)
            g_bf = pool_long.tile([8, 4096], BF, name="g_bf")
            vp_bf = pool_long.tile([8, 4096], BF, name="vp_bf")
            rmaxk = pool_ev.tile([8, 64, 128], BF, tag="rmax", name="rmaxk", bufs=1)
            rmaxv = rmaxk

            def warm_mm():
                jp = psc.tile([128, 512], F32, tag="warm", name="warmps", bufs=1)
                nc.tensor.matmul(jp[:, :], inp_sb[0:8, 0:128], inp_sb[0:8, 0:512],
                                 start=True, stop=True)

            for (plane, rmax, dst, eng) in ((kpl, rmaxk, g_bf, nc.vector),
                                            (vpl, rmaxv, vp_bf, nc.vector)):
                pv = plane.rearrange("c (y ty) x -> c y ty x", ty=2)
                eng.tensor_tensor(out=rmax, in0=pv[:, :, 0, :], in1=pv[:, :, 1, :],
                                  op=ALU.max)
                rv = rmax.rearrange("c y (x tx) -> c y x tx", tx=2)
                o = dst.rearrange("c (y x) -> c y x", y=64)
                eng.tensor_tensor(out=o, in0=rv[:, :, :, 0], in1=rv[:, :, :, 1],
                                  op=ALU.max)
                warm_mm()

            # phi transpose -> phi_sb [128, 32, 9] (col 8 = ones)
            phi_sb = pool_long.tile([128, 32, 9], BF, name="phi_sb")
            nc.vector.memset(phi_sb, 1.0)
            for kt in range(32):
                pst_ = psc.tile([128, 8], BF, tag="cps", name="tps")
                nc.tensor.transpose(pst_[:, :], vp_bf[0:8, 128 * kt:128 * kt + 128],
                                    ident8[:, :])
                nc.vector.tensor_copy(out=phi_sb[:, kt, 0:8], in_=pst_)
                if kt % 8 == 7:
                    warm_mm()

            # stage inp to DRAM with zero border rows
            for rr in (0, 1, 130, 131, 132):
                nc.sync.dma_start(out=T["stage"][:, rr:rr + 1, :], in_=zrow[:, :])
            nc.sync.dma_start(out=T["stage"][:, 2:130, :], in_=inp_sb[:, :])

        if phases == 3:
            nc.gpsimd.dma_start(out=T["A"][:, 0:4096], in_=g_bf[0:4, :])
            return

        # ---- local-strip phase ----
        pool_attn = ctx.enter_context(tc.tile_pool(name="attn", bufs=1))
        mask_rep = pool_attn.tile([8, 37, 128], F32, name="mask_rep")
        nc.sync.dma_start(out=mask_rep, in_=T["mask"][:, :, :])

        strips32 = pool_attn.tile([32, 37, 128], BF, name="strips32")
        for r in range(4):
            nc.sync.dma_start(out=strips32[8 * r:8 * r + 8, :, :],
                              in_=T["stage"][:, 32 * r:32 * r + 37, :])

        inpl = pool_attn.tile([48, 42, 133], BF, name="inpl")
        _zero_plane_borders(nc, inpl, 42, 133, 2, 3, 2, 6, 8)
        f_sb = pool_attn.tile([8, 4736], BF, name="f_sb")
        aps = pool_attn.tile([9, 4736], F32, name="aps")
        lcm = pool_attn.tile([48, 42, 133], BF, name="lcm")
        _zero_plane_borders(nc, lcm, 42, 133, 2, 3, 2, 6, 8)

        with tc.tile_pool(name="psf", bufs=3, space="PSUM") as psf:
            def warm_mm2():
                jp = psf.tile([128, 512], F32, tag="warm", name="warmps2", bufs=1)
                nc.tensor.matmul(jp[:, :], strips32[0:8, 0, :],
                                 strips32[0:8, 0:4, :], start=True, stop=True)
            # one-hot strip select (also 6-dup for the conv6 i2c plane)
            selb = pool_attn.tile([8, 37, 128], BF, name="selb")
            for t in range(10):
                nr = 4 if t < 9 else 1
                ps = psf.tile([8, 4, 128], F32, tag="fps", name="selps")
                nc.tensor.matmul(ps[:, 0:nr, :], sel4[:, :],
                                 strips32[:, 4 * t:4 * t + nr, :],
                                 start=True, stop=True)
                nc.vector.tensor_copy(out=selb[:, 4 * t:4 * t + nr, :],
                                      in_=ps[:, 0:nr, :])
                if t % 3 == 2:
                    warm_mm2()
            _i2c_flush(nc, inpl, selb, 8, 6, 0, 37, 2)
            # f = Wq . inp_loc
            for t in range(10):
                nr = 4 if t < 9 else 1
                ps = psf.tile([8, 4, 128], F32, tag="fps", name="fps")
                nc.tensor.matmul(ps[:, 0:nr, :], Wq[:, :],
                                 inpl[0:8, 2 + 4 * t:2 + 4 * t + nr, 2:130],
                                 start=True, stop=True)
                nc.vector.tensor_copy(out=f_sb[:, 512 * t:512 * t + 128 * nr],
                                      in_=ps[:, 0:nr, :])
                if t % 3 == 1:
                    warm_mm2()

        if phases == 4:
            nc.sync.dma_start(out=T["A"][:, :], in_=f_sb[0:4, :])
            return

        # ---- attention ----
        QCH = [(0, 1024), (1024, 1024), (2048, 1024), (3072, 1024), (4096, 640)]
        with tc.tile_pool(name="pse", bufs=2) as pool_e, \
             tc.tile_pool(name="pss", bufs=2, space="PSUM") as pss, \
             tc.tile_pool(name="psap", bufs=2, space="PSUM") as psap:
            for (qoff, cw) in QCH:
                ap_ps = psap.tile([9, 1024], F32, name="ap_ps")
                nsub = (cw + 511) // 512
                for kt in range(32):
                    s_ps = pss.tile([128, 1024], F32, name="s_ps")
                    for sub in range(nsub):
                        w = min(512, cw - 512 * sub)
                        nc.tensor.matmul(s_ps[:, 512 * sub:512 * sub + w],
                                         g_bf[0:8, 128 * kt:128 * kt + 128],
                                         f_sb[0:8, qoff + 512 * sub:qoff + 512 * sub + w],
                                         start=True, stop=True)
                    e_sb = pool_e.tile([128, 1024], BF, name="e_sb")
                    nc.scalar.activation(out=e_sb[:, 0:cw], in_=s_ps[:, 0:cw],
                                         func=AF.Exp, bias=0.0)
                    for sub in range(nsub):
                        w = min(512, cw - 512 * sub)
                        nc.tensor.matmul(ap_ps[:, 512 * sub:512 * sub + w],
                                         phi_sb[:, kt, :],
                                         e_sb[:, 512 * sub:512 * sub + w],
                                         start=(kt == 0), stop=(kt == 31))
                nc.vector.tensor_copy(out=aps[:, qoff:qoff + cw], in_=ap_ps[:, 0:cw])

        if phases == 5:
            nc.sync.dma_start(out=T["A"][:, :], in_=aps[0:4, :])
            nc.sync.dma_start(out=T["z"][0:1, :], in_=aps[8:9, :])
            return

        # ---- tail ----
        ap_bf = pool_attn.tile([8, 4736], BF, name="ap_bf")
        nc.vector.tensor_copy(out=ap_bf, in_=aps[0:8, :])
        with tc.tile_pool(name="pst", bufs=3, space="PSUM") as pst:
            ap3 = ap_bf.rearrange("c (y x) -> c y x", x=128)
            lcb = pool_attn.tile([8, 37, 128], BF, name="lcb")
            for t in range(10):
                nr = 4 if t < 9 else 1
                ps = pst.tile([8, 4, 128], F32, tag="tailps", name="lcps")
                nc.tensor.matmul(ps[:, 0:nr, :], Wl[:, :],
                                 ap3[:, 4 * t:4 * t + nr, :],
                                 start=True, stop=True)
                nc.vector.tensor_tensor(out=lcb[:, 4 * t:4 * t + nr, :],
                                        in0=ps[:, 0:nr, :],
                                        in1=mask_rep[:, 4 * t:4 * t + nr, :],
                                        op=ALU.mult)
            _i2c_flush(nc, lcm, lcb, 8, 6, 0, 37, 2)
            nc.sync.dma_start(out=T["z"][0:1, :], in_=aps[8:9, :])

            for (plane, out_d, nm) in ((inpl, T["A"], "A"), (lcm, T["Bo"], "B")):
                ob = pool_attn.tile([4, 37, 128], F32, name=f"ob{nm}")
                for t in range(10):
                    nr = 4 if t < 9 else 1
                    ps = pst.tile([4, 4, 128], F32, tag="tailps", name=f"c6{nm}")
                    for dy in range(6):
                        nc.tensor.matmul(ps[:, 0:nr, :], W6[:, dy, :],
                                         plane[0:48, 4 * t + dy:4 * t + dy + nr, 0:128],
                                         start=(dy == 0), stop=(dy == 5))
                    nc.vector.tensor_copy(out=ob[:, 4 * t:4 * t + nr, :],
                                          in_=ps[:, 0:nr, :])
                nc.sync.dma_start(out=out_d[:, :], in_=ob[:, :, :])


# ------------------------------------------------------------- host side --
def host_prep(x, p):
    def n(a):
        return np.asarray(a, dtype=np.float32)

    B = x.shape[0]
    g1, be1, m1, v1 = (n(a) for a in p['bn1'])
    xhat = g1 * (n(x) - m1) / np.sqrt(v1 + EPS) + be1

    W1 = n(p['cb1_w'])
    lhs1 = np.zeros((128, 40), np.float32)
    Pms = []
    for b in range(B):
        xpad = np.zeros((4, 135, 135), np.float32)
        xpad[:, 3:131, 3:131] = xhat[b].transpose(2, 0, 1)
        Pb = np.zeros((128, 132, 128), np.float32)
        for dy in range(4):
            for dx in range(8):
                for c in range(4):
                    prt = dy * 32 + dx * 4 + c
                    Pb[prt] = xpad[c, dy:dy + 132, dx:dx + 128]
                    lhs1[prt, 0:20] = W1[dy, dx, c]
                    lhs1[prt, 20:40] = W1[dy + 4, dx, c]
        Pms.append(Pb.reshape(128, 16896))

    def bn_affine(key):
        g, bt, mu, va = (n(a) for a in p[key])
        s = g / np.sqrt(va + EPS)
        return s.astype(np.float32), (bt - mu * s).astype(np.float32)

    s2, t2 = bn_affine('bn2')
    s3, t3 = bn_affine('bn3')
    s4, t4 = bn_affine('bn4')

    def col(v):
        return np.ascontiguousarray(np.asarray(v, np.float32).reshape(-1, 1))

    def pack_w(Wc, cin, cout, chunks):
        out = np.zeros((128, len(chunks), cout), np.float32)
        for c, (dy, dxe, kp) in enumerate(chunks):
            ngr = kp // cin
            for qi in range(ngr):
                out[cin * qi:cin * (qi + 1), c, :] = Wc[dy, qi + dxe]
        return out

    ch6 = [(d, 0, 48) for d in range(6)]
    W2sb = pack_w(n(p['cb2_w']), 20, 30, CH2)
    W3sb = pack_w(n(p['cb3_w']), 30, 40, CH3)
    W4sb = pack_w(n(p['cb4_w']), 40, 50, CH4)
    Wapc = np.concatenate([n(p['amp_w']), PI * n(p['phase_w'])], axis=3)
    Wapsb = pack_w(Wapc, 50, 8, CHAP)
    W6sb = pack_w(n(p['out_w']), 8, 4, ch6)[0:48]

    bf = lambda a: np.asarray(a).astype(ml_dtypes.bfloat16)
    shared = dict(
        lhs1=bf(lhs1),
        b1=col(n(p['cb1_b'])), s2=col(s2), t2=col(t2),
        W2sb=bf(W2sb), b2=col(n(p['cb2_b'])), s3=col(s3), t3=col(t3),
        W3sb=bf(W3sb), b3=col(n(p['cb3_b'])), s4=col(s4), t4=col(t4),
        W4sb=bf(W4sb), b4=col(n(p['cb4_b'])),
        Wap=bf(Wapsb),
        Wk=bf(n(p['k_w'])[0, 0]), Wv=bf(n(p['v_w'])[0, 0]), Wq=bf(n(p['q_w'])[0, 0]),
        Wl=bf(n(p['last_w'])[0, 0]), W6sb=bf(W6sb),
    )

    in_maps = []
    for core in range(8):
        b, r = core // 4, core % 4
        mask = np.ones((4736,), np.float32)
        if r == 0:
            mask[0:256] = 0
        if r == 3:
            mask[34 * 128:] = 0
        mask = np.broadcast_to(mask.reshape(1, 37, 128), (8, 37, 128)).copy()
        sel4 = np.zeros((32, 8), np.float32)
        for c in range(8):
            sel4[8 * r + c, c] = 1.0
        sel4 = sel4.astype(ml_dtypes.bfloat16)
        m = dict(shared)
        m['P'] = Pms[b].astype(ml_dtypes.bfloat16)
        m['mask'] = mask
        m['sel4'] = sel4
        in_maps.append(m)
    return in_maps


def host_assemble(results, sigma):
    out = np.zeros((2, 128, 128, 4), np.float32)
    for b in range(2):
        Z = np.float64(0)
        for r in range(4):
            Z += results[4 * b + r]['z'][0, 256:4352].sum(dtype=np.float64)
        Z = np.float32(Z)
        for r in range(4):
            res = results[4 * b + r]
            strip = res['A'][:, 256:4352] + (np.float32(sigma) / Z) * res['Bo'][:, 256:4352]
            out[b, 32 * r:32 * r + 32] = strip.reshape(4, 32, 128).transpose(1, 2, 0)
    return out


def kernel(x, params):
    x = np.asarray(x)
    if 'nc' not in _NC_CACHE:
        _NC_CACHE['nc'] = build_nc()
    nc = _NC_CACHE['nc']
    in_maps = host_prep(x, params)
    res = run_bass_kernel_spmd(nc, in_maps, core_ids=list(range(8)))
    return host_assemble(res.results, np.asarray(params['sigma']))


if __name__ == '__main__':
    nc = build_nc()
    print("built ok; instructions:",
          sum(len(bb.instructions) for fn in nc.m.functions for bb in fn.blocks))


# revision 35
# speedup vs baseline: 1.1779x; 1.1779x over previous
"""Trainium2 Bass kernel for nn_AttentionBlockConv_55121610276882.

Strategy (8 cores, full I/O):
  core c -> (batch b = c//4, query-strip r = c%4).
  Each core: full ConvBlock for its batch image, k/v convs + 2x2 maxpool for
  the full key set, then flash-style attention for its 4736-query window
  (37 rows = 32-row strip + conv6 halo).  The joint softmax uses exp(s)
  directly (M=0 is safe: |s| < 1); the global normalizer Z is finished on
  the host.  The tail is decomposed linearly:
      out = conv6(inp) + (sigma/Z) * conv6(W_last * ap_unnorm)
  so each core returns A = conv6(inp_strip), B = conv6(lc_strip), z-partials.

  Convs run as K-packed matmuls WITHOUT tile_position (which faults on this
  runtime): the input plane is stored in Q dx-shifted partition groups
  (group q holds the plane shifted left by q columns), so one matmul
  contracts over (dx-group x channel) = K<=120 and the dy taps become
  free-dim row offsets chunked over accumulating matmuls.  The group
  replication is free: the producing conv's lhsT duplicates its output
  columns Q times, and the PSUM->SBUF eviction writes each group at its
  shifted column range (same partitions, so DVE/ACT can do it).

  Per-core divergence (which 37-row query window) is data-driven: a
  per-core one-hot selection matrix picks 1 of 4 statically-gathered strips
  via a matmul, so all 8 cores run an identical program.
"""
import sys

sys.path.insert(0, '/opt/trn_rl_repo')

import numpy as np
import ml_dtypes
import concourse.bass as bass
import concourse.mybir as mybir
import concourse.tile as tile
from concourse.bass_utils import run_bass_kernel_spmd
from concourse.masks import make_identity

F32 = mybir.dt.float32
BF = mybir.dt.bfloat16
AF = mybir.ActivationFunctionType
ALU = mybir.AluOpType

EPS = 1e-3
PI = np.float32(np.pi)

_NC_CACHE = {}


def split_multiwaits(nc, max_waits=1):
    """walrus here rejects >1 sync-wait per instruction (tail Drain).
    Hoist extra waits onto dedicated NOPs."""
    n_new = 0
    for fn in nc.m.functions:
        for bb in fn.blocks:
            insts = bb.instructions
            i = 0
            while i < len(insts):
                inst = insts[i]
                si = inst.sync_info
                if si is not None and len(si.on_wait) > max_waits:
                    waits = list(si.on_wait)
                    keep = waits[-max_waits:]
                    extra = waits[:-max_waits]
                    nops = []
                    for w in extra:
                        nops.append(mybir.InstNoOp(
                            name=f"{inst.name}-ws{n_new}",
                            engine=inst.engine,
                            ins=[], outs=[],
                            sync_info=mybir.SyncInfo(on_wait=[w], on_update=[]),
                        ))
                        n_new += 1
                    inst.sync_info = mybir.SyncInfo(on_wait=keep,
                                                    on_update=list(si.on_update))
                    for j, nop in enumerate(nops):
                        insts.insert(i + j, nop)
                    i += len(nops)
                i += 1
    return n_new


# chunk lists: (dy, dx_extra, kparts)
CH2 = [(d, 0, 100) for d in range(5)]
CH3 = [(d, 0, 120) for d in range(5)] + [(d, 4, 30) for d in range(5)]
CH4 = [(d, 0, 120) for d in range(5)] + [(d, 3, 80) for d in range(5)]
CHAP = [(d, 0, 100) for d in range(3)] + [(d, 2, 50) for d in range(3)]


def build_nc(for_hw=True, phases=6):
    nc = bass.Bass("TRN2")

    io = {}

    def dinp(name, shape, dt=F32):
        io[name] = nc.dram_tensor(name, shape, dt, kind="ExternalInput")

    dinp("P", [128, 16896], BF)
    dinp("lhs1", [128, 40], BF)
    dinp("b1", [20, 1]); dinp("s2", [20, 1]); dinp("t2", [20, 1])
    dinp("W2sb", [128, len(CH2), 30], BF)
    dinp("b2", [30, 1]); dinp("s3", [30, 1]); dinp("t3", [30, 1])
    dinp("W3sb", [128, len(CH3), 40], BF)
    dinp("b3", [40, 1]); dinp("s4", [40, 1]); dinp("t4", [40, 1])
    dinp("W4sb", [128, len(CH4), 50], BF)
    dinp("b4", [50, 1])
    dinp("Wap", [128, len(CHAP), 8], BF)
    dinp("Wk", [8, 8], BF); dinp("Wv", [8, 8], BF); dinp("Wq", [8, 8], BF)
    dinp("Wl", [8, 8], BF)
    dinp("W6sb", [48, 6, 4], BF)
    dinp("sel4", [32, 8], BF)
    dinp("mask", [8, 37, 128])

    io["A"] = nc.dram_tensor("A", [4, 4736], F32, kind="ExternalOutput")
    io["Bo"] = nc.dram_tensor("Bo", [4, 4736], F32, kind="ExternalOutput")
    io["z"] = nc.dram_tensor("z", [1, 4736], F32, kind="ExternalOutput")
    io["stage"] = nc.dram_tensor("stage", [8, 133, 128], BF, kind="Internal")

    with tile.TileContext(nc) as tc:
        _emit(nc, tc, io, phases)

    if for_hw:
        split_multiwaits(nc)
    return nc


_DMA_ENGS = None


def _dma_rr(nc):
    """Round-robin DMA issue across engine sequencers."""
    global _DMA_ENGS
    if _DMA_ENGS is None or _DMA_ENGS[0] is not nc:
        _DMA_ENGS = (nc, [nc.sync, nc.scalar, nc.gpsimd], [0])
    engs, ctr = _DMA_ENGS[1], _DMA_ENGS[2]
    e = engs[ctr[0] % len(engs)]
    ctr[0] += 1
    return e


def _i2c_flush(nc, plane, batch, cch, Q, t0, nrows, padl):
    """DMA the batch scratch [cch, nrows, 128] into Q shifted plane groups;
    batch rows correspond to plane rows padl+t0*4 .. +nrows."""
    for q in range(Q):
        lo = padl - q
        srcl = 0
        if lo < 0:
            srcl = -lo
            lo = 0
        width = 128 - srcl
        dsl = plane[cch * q:cch * (q + 1),
                    padl + 4 * t0:padl + 4 * t0 + nrows, lo:lo + width]
        ssl = batch[0:cch, 0:nrows, srcl:128]
        _dma_rr(nc).dma_start(out=dsl, in_=ssl)


def _zero_plane_borders(nc, plane, H, W, prl, prh, pcl, Q, cch):
    """Row borders, left cols, and per-group right/unwritten column ranges."""
    nc.gpsimd.memset(plane[:, 0:prl, :], 0.0)
    nc.gpsimd.memset(plane[:, H - prh:H, :], 0.0)
    if pcl > 0:
        nc.gpsimd.memset(plane[:, :, 0:pcl], 0.0)
    st = pcl + 128 - (Q - 1)
    if st < W:
        nc.gpsimd.memset(plane[:, :, st:W], 0.0)


def _emit(nc, tc, T, phases=6):
    from contextlib import ExitStack
    ctx = ExitStack()
    with ctx:
        pool_w = ctx.enter_context(tc.tile_pool(name="weights", bufs=1))
        pool_long = ctx.enter_context(tc.tile_pool(name="long", bufs=1))
        pool_ev = ctx.enter_context(tc.tile_pool(name="evict", bufs=2))

        def wtile(name, shape, dt=F32):
            t = pool_w.tile(shape, dt, name=f"w_{name}")
            nc.sync.dma_start(out=t, in_=T[name][tuple(slice(None) for _ in shape)])
            return t

        def wconv(name, shape):
            t = pool_w.tile(shape, BF, tag="wconv", name=f"w_{name}")
            nc.sync.dma_start(out=t, in_=T[name][tuple(slice(None) for _ in shape)])
            return t

        lhs1 = wconv("lhs1", [128, 40])
        b1 = wtile("b1", [20, 1]); s2 = wtile("s2", [20, 1]); t2 = wtile("t2", [20, 1])
        W2 = wconv("W2sb", [128, len(CH2), 30])
        b2 = wtile("b2", [30, 1]); s3 = wtile("s3", [30, 1]); t3 = wtile("t3", [30, 1])
        W3 = wconv("W3sb", [128, len(CH3), 40])
        b3 = wtile("b3", [40, 1]); s4 = wtile("s4", [40, 1]); t4 = wtile("t4", [40, 1])
        W4 = wconv("W4sb", [128, len(CH4), 50])
        b4 = wtile("b4", [50, 1])
        Wap = wtile("Wap", [128, len(CHAP), 8], BF)
        Wk = wtile("Wk", [8, 8], BF); Wv = wtile("Wv", [8, 8], BF); Wq = wtile("Wq", [8, 8], BF)
        Wl = wtile("Wl", [8, 8], BF)
        W6 = wtile("W6sb", [48, 6, 4], BF)
        sel4 = wtile("sel4", [32, 8], BF)
        ident8 = pool_w.tile([8, 8], BF, name="ident8")
        make_identity(nc, ident8)
        zrow = pool_w.tile([8, 128], BF, name="zrow")
        nc.vector.memset(zrow, 0.0)

        # ---- conv phase ----
        with tc.tile_pool(name="planes", bufs=1) as pool_pl, \
             tc.tile_pool(name="psc", bufs=4, space="PSUM") as psc:
            P_sb = pool_pl.tile([128, 16896], BF, tag="T1", name="P_sb")
            for i in range(4):
                nc.sync.dma_start(out=P_sb[:, i * 4224:(i + 1) * 4224],
                                  in_=T["P"][:, i * 4224:(i + 1) * 4224])
            h1p = pool_pl.tile([100, 132, 132], BF, tag="T2", name="h1p")
            _zero_plane_borders(nc, h1p, 132, 132, 2, 2, 2, 5, 20)

            batch = None
            for t in range(32):
                if phases < 1:
                    break
                ps = psc.tile([20, 4, 128], F32, tag="cps", name="c1ps")
                nc.tensor.matmul(ps[:, :, :], lhs1[:, 0:20],
                                 P_sb[:, 512 * t:512 * t + 512],
                                 start=True, stop=False)
                nc.tensor.matmul(ps[:, :, :], lhs1[:, 20:40],
                                 P_sb[:, 512 * t + 512:512 * t + 1024],
                                 start=False, stop=True)
                sc = pool_ev.tile([20, 4, 128], F32, tag="ev", name="c1sc")
                nc.vector.tensor_scalar(out=sc, in0=ps, scalar1=b1[:, 0:1],
                                        scalar2=0.0, op0=ALU.add, op1=ALU.max)
                if t % 4 == 0:
                    batch = pool_ev.tile([20, 16, 128], BF, tag="bat", name="c1b")
                nc.gpsimd.tensor_scalar(out=batch[:, 4 * (t % 4):4 * (t % 4) + 4, :],
                                        in0=sc, scalar1=s2[:, 0:1], scalar2=t2[:, 0:1],
                                        op0=ALU.mult, op1=ALU.add)
                if t % 4 == 3:
                    _i2c_flush(nc, h1p, batch, 20, 5, t - 3, 16, 2)

            if phases == 1:
                nc.sync.dma_start(out=T["A"][:, :], in_=h1p[0:4, 2:39, 2:130])
                return

            def tap_conv(src, Wsb, chunks, cout, bias, affine, dstplane, Q, padl):
                batch = None
                for t in range(32):
                    ps = psc.tile([cout, 4, 128], F32, tag="cps",
                                  name=f"cps{cout}")
                    for c, (dy, dxe, kp) in enumerate(chunks):
                        nc.tensor.matmul(ps[:, :, :], Wsb[0:kp, c, :],
                                         src[0:kp, 4 * t + dy:4 * t + dy + 4,
                                             dxe:dxe + 128],
                                         start=(c == 0), stop=(c == len(chunks) - 1))
                    if t % 4 == 0:
                        batch = pool_ev.tile([cout, 16, 128], BF, tag="bat",
                                             name=f"b{cout}")
                    bsl = batch[:, 4 * (t % 4):4 * (t % 4) + 4, :]
                    if affine is not None:
                        sc = pool_ev.tile([cout, 4, 128], F32, tag="ev",
                                          name=f"sc{cout}")
                        nc.vector.tensor_scalar(out=sc, in0=ps, scalar1=bias[:, 0:1],
                                                scalar2=0.0, op0=ALU.add, op1=ALU.max)
                        nc.gpsimd.tensor_scalar(out=bsl, in0=sc,
                                                scalar1=affine[0][:, 0:1],
                                                scalar2=affine[1][:, 0:1],
                                                op0=ALU.mult, op1=ALU.add)
                    else:
                        nc.vector.tensor_scalar(out=bsl, in0=ps, scalar1=bias[:, 0:1],
                                                scalar2=0.0, op0=ALU.add, op1=ALU.max)
                    if t % 4 == 3:
                        _i2c_flush(nc, dstplane, batch, cout, Q, t - 3, 16, padl)

            h2p = pool_pl.tile([120, 132, 132], BF, tag="T1", name="h2p")
            _zero_plane_borders(nc, h2p, 132, 132, 2, 2, 2, 4, 30)
            tap_conv(h1p, W2, CH2, 30, b2, (s3, t3), h2p, 4, 2)

            h3p = pool_pl.tile([120, 132, 132], BF, tag="T2", name="h3p")
            _zero_plane_borders(nc, h3p, 132, 132, 2, 2, 2, 3, 40)
            tap_conv(h2p, W3, CH3, 40, b3, (s4, t4), h3p, 3, 2)

            h4p = pool_pl.tile([100, 130, 130], BF, tag="T1", name="h4p")
            _zero_plane_borders(nc, h4p, 130, 130, 1, 1, 1, 2, 50)
            tap_conv(h3p, W4, CH4, 50, b4, None, h4p, 2, 1)

            # amp/phase -> inp_sb [8, 16384] (amp relu, phase plain, pi folded)
            inp_sb = pool_pl.tile([8, 16384], BF, tag="T2", name="inp_sb")
            for t in range(32):
                ps = psc.tile([8, 4, 128], F32, tag="cps", name="apps")
                for c, (dy, dxe, kp) in enumerate(CHAP):
                    nc.tensor.matmul(ps[:, :, :], Wap[0:kp, c, :],
                                     h4p[0:kp, 4 * t + dy:4 * t + dy + 4, dxe:dxe + 128],
                                     start=(c == 0), stop=(c == len(CHAP) - 1))
                nc.vector.tensor_copy(out=inp_sb[0:8, 512 * t:512 * t + 512], in_=ps[0:8])
                nc.scalar.activation(out=inp_sb[0:4, 512 * t:512 * t + 512],
                                     in_=ps[0:4], func=AF.Relu, bias=0.0)

            if phases == 2:
                nc.sync.dma_start(out=T["A"][:, :], in_=inp_sb[0:4, 0:4736])
                return

            # k and v 1x1 convs into bf16 planes, then batched 2x2 maxpool
            kpl = pool_pl.tile([8, 128, 128], BF, tag="kvk", name="kpl")
            vpl = pool_pl.tile([8, 128, 128], BF, tag="kvv", name="vpl")
            for t in range(32):
                for ei, (Wt, dst) in enumerate(((Wk, kpl), (Wv, vpl))):
                    psk = psc.tile([8, 4, 128], F32, tag="cps", name="kvps")
                    nc.tensor.matmul(psk[:, :, :], Wt[:, :],
                                     inp_sb[:, 512 * t:512 * t + 512],
                                     start=True, stop=True)
                    nc.vector.tensor_copy(out=dst[:, 4 * t:4 * t + 4, :], in_=psk)
            g_bf = pool_long.tile([8, 4096], BF, name="g_bf")
            vp_bf = pool_long.tile([8, 4096], BF, name="vp_bf")
            rmaxk = pool_ev.tile([8, 64, 128], BF, tag="rmax", name="rmaxk", bufs=1)
            rmaxv = rmaxk

            def warm_mm():
                pass

            for (plane, rmax, dst, eng) in ((kpl, rmaxk, g_bf, nc.vector),
                                            (vpl, rmaxv, vp_bf, nc.vector)):
                pv = plane.rearrange("c (y ty) x -> c y ty x", ty=2)
                eng.tensor_tensor(out=rmax, in0=pv[:, :, 0, :], in1=pv[:, :, 1, :],
                                  op=ALU.max)
                rv = rmax.rearrange("c y (x tx) -> c y x tx", tx=2)
                o = dst.rearrange("c (y x) -> c y x", y=64)
                eng.tensor_tensor(out=o, in0=rv[:, :, :, 0], in1=rv[:, :, :, 1],
                                  op=ALU.max)
                warm_mm()

            # phi transpose -> phi_sb [128, 32, 9] (col 8 = ones)
            phi_sb = pool_long.tile([128, 32, 9], BF, name="phi_sb")
            nc.vector.memset(phi_sb, 1.0)
            for kt in range(32):
                pst_ = psc.tile([128, 8], BF, tag="cps", name="tps")
                nc.tensor.transpose(pst_[:, :], vp_bf[0:8, 128 * kt:128 * kt + 128],
                                    ident8[:, :])
                nc.vector.tensor_copy(out=phi_sb[:, kt, 0:8], in_=pst_)
                if kt % 8 == 7:
                    warm_mm()

            # stage inp to DRAM with zero border rows
            for rr in (0, 1, 130, 131, 132):
                nc.sync.dma_start(out=T["stage"][:, rr:rr + 1, :], in_=zrow[:, :])
            nc.sync.dma_start(out=T["stage"][:, 2:130, :], in_=inp_sb[:, :])

        if phases == 3:
            nc.gpsimd.dma_start(out=T["A"][:, 0:4096], in_=g_bf[0:4, :])
            return

        # ---- local-strip phase ----
        pool_attn = ctx.enter_context(tc.tile_pool(name="attn", bufs=1))
        mask_rep = pool_attn.tile([8, 37, 128], F32, name="mask_rep")
        nc.sync.dma_start(out=mask_rep, in_=T["mask"][:, :, :])

        strips32 = pool_attn.tile([32, 37, 128], BF, name="strips32")
        for r in range(4):
            nc.sync.dma_start(out=strips32[8 * r:8 * r + 8, :, :],
                              in_=T["stage"][:, 32 * r:32 * r + 37, :])

        inpl = pool_attn.tile([48, 42, 133], BF, name="inpl")
        _zero_plane_borders(nc, inpl, 42, 133, 2, 3, 2, 6, 8)
        f_sb = pool_attn.tile([8, 4736], BF, name="f_sb")
        aps = pool_attn.tile([9, 4736], F32, name="aps")
        lcm = pool_attn.tile([48, 42, 133], BF, name="lcm")
        _zero_plane_borders(nc, lcm, 42, 133, 2, 3, 2, 6, 8)

        with tc.tile_pool(name="psf", bufs=3, space="PSUM") as psf:
            def warm_mm2():
                pass
            # one-hot strip select (also 6-dup for the conv6 i2c plane)
            selb = pool_attn.tile([8, 37, 128], BF, name="selb")
            for t in range(10):
                nr = 4 if t < 9 else 1
                ps = psf.tile([8, 4, 128], F32, tag="fps", name="selps")
                nc.tensor.matmul(ps[:, 0:nr, :], sel4[:, :],
                                 strips32[:, 4 * t:4 * t + nr, :],
                                 start=True, stop=True)
                nc.vector.tensor_copy(out=selb[:, 4 * t:4 * t + nr, :],
                                      in_=ps[:, 0:nr, :])
                if t % 3 == 2:
                    warm_mm2()
            _i2c_flush(nc, inpl, selb, 8, 6, 0, 37, 2)
            # f = Wq . inp_loc
            for t in range(10):
                nr = 4 if t < 9 else 1
                ps = psf.tile([8, 4, 128], F32, tag="fps", name="fps")
                nc.tensor.matmul(ps[:, 0:nr, :], Wq[:, :],
                                 inpl[0:8, 2 + 4 * t:2 + 4 * t + nr, 2:130],
                                 start=True, stop=True)
                nc.vector.tensor_copy(out=f_sb[:, 512 * t:512 * t + 128 * nr],
                                      in_=ps[:, 0:nr, :])
                if t % 3 == 1:
                    warm_mm2()

        if phases == 4:
            nc.sync.dma_start(out=T["A"][:, :], in_=f_sb[0:4, :])
            return

        # ---- attention ----
        QCH = [(0, 1024), (1024, 1024), (2048, 1024), (3072, 1024), (4096, 640)]
        with tc.tile_pool(name="pse", bufs=2) as pool_e, \
             tc.tile_pool(name="pss", bufs=2, space="PSUM") as pss, \
             tc.tile_pool(name="psap", bufs=2, space="PSUM") as psap:
            for (qoff, cw) in QCH:
                ap_ps = psap.tile([9, 1024], F32, name="ap_ps")
                nsub = (cw + 511) // 512
                for kt in range(32):
                    s_ps = pss.tile([128, 1024], F32, name="s_ps")
                    for sub in range(nsub):
                        w = min(512, cw - 512 * sub)
                        nc.tensor.matmul(s_ps[:, 512 * sub:512 * sub + w],
                                         g_bf[0:8, 128 * kt:128 * kt + 128],
                                         f_sb[0:8, qoff + 512 * sub:qoff + 512 * sub + w],
                                         start=True, stop=True)
                    e_sb = pool_e.tile([128, 1024], BF, name="e_sb")
                    nc.scalar.activation(out=e_sb[:, 0:cw], in_=s_ps[:, 0:cw],
                                         func=AF.Exp, bias=0.0)
                    for sub in range(nsub):
                        w = min(512, cw - 512 * sub)
                        nc.tensor.matmul(ap_ps[:, 512 * sub:512 * sub + w],
                                         phi_sb[:, kt, :],
                                         e_sb[:, 512 * sub:512 * sub + w],
                                         start=(kt == 0), stop=(kt == 31))
                nc.vector.tensor_copy(out=aps[:, qoff:qoff + cw], in_=ap_ps[:, 0:cw])

        if phases == 5:
            nc.sync.dma_start(out=T["A"][:, :], in_=aps[0:4, :])
            nc.sync.dma_start(out=T["z"][0:1, :], in_=aps[8:9, :])
            return

        # ---- tail ----
        ap_bf = pool_attn.tile([8, 4736], BF, name="ap_bf")
        nc.vector.tensor_copy(out=ap_bf, in_=aps[0:8, :])
        with tc.tile_pool(name="pst", bufs=3, space="PSUM") as pst:
            ap3 = ap_bf.rearrange("c (y x) -> c y x", x=128)
            lcb = pool_attn.tile([8, 37, 128], BF, name="lcb")
            for t in range(10):
                nr = 4 if t < 9 else 1
                ps = pst.tile([8, 4, 128], F32, tag="tailps", name="lcps")
                nc.tensor.matmul(ps[:, 0:nr, :], Wl[:, :],
                                 ap3[:, 4 * t:4 * t + nr, :],
                                 start=True, stop=True)
                nc.vector.tensor_tensor(out=lcb[:, 4 * t:4 * t + nr, :],
                                        in0=ps[:, 0:nr, :],
                                        in1=mask_rep[:, 4 * t:4 * t + nr, :],
                                        op=ALU.mult)
            _i2c_flush(nc, lcm, lcb, 8, 6, 0, 37, 2)
            nc.sync.dma_start(out=T["z"][0:1, :], in_=aps[8:9, :])

            for (plane, out_d, nm) in ((inpl, T["A"], "A"), (lcm, T["Bo"], "B")):
                ob = pool_attn.tile([4, 37, 128], F32, name=f"ob{nm}")
                for t in range(10):
                    nr = 4 if t < 9 else 1
                    ps = pst.tile([4, 4, 128], F32, tag="tailps", name=f"c6{nm}")
                    for dy in range(6):
                        nc.tensor.matmul(ps[:, 0:nr, :], W6[:, dy, :],
                                         plane[0:48, 4 * t + dy:4 * t + dy + nr, 0:128],
                                         start=(dy == 0), stop=(dy == 5))
                    nc.vector.tensor_copy(out=ob[:, 4 * t:4 * t + nr, :],
                                          in_=ps[:, 0:nr, :])
                nc.sync.dma_start(out=out_d[:, :], in_=ob[:, :, :])


# ------------------------------------------------------------- host side --
def host_prep(x, p):
    def n(a):
        return np.asarray(a, dtype=np.float32)

    B = x.shape[0]
    g1, be1, m1, v1 = (n(a) for a in p['bn1'])
    xhat = g1 * (n(x) - m1) / np.sqrt(v1 + EPS) + be1

    W1 = n(p['cb1_w'])
    lhs1 = np.zeros((128, 40), np.float32)
    Pms = []
    for b in range(B):
        xpad = np.zeros((4, 135, 135), np.float32)
        xpad[:, 3:131, 3:131] = xhat[b].transpose(2, 0, 1)
        Pb = np.zeros((128, 132, 128), np.float32)
        for dy in range(4):
            for dx in range(8):
                for c in range(4):
                    prt = dy * 32 + dx * 4 + c
                    Pb[prt] = xpad[c, dy:dy + 132, dx:dx + 128]
                    lhs1[prt, 0:20] = W1[dy, dx, c]
                    lhs1[prt, 20:40] = W1[dy + 4, dx, c]
        Pms.append(Pb.reshape(128, 16896))

    def bn_affine(key):
        g, bt, mu, va = (n(a) for a in p[key])
        s = g / np.sqrt(va + EPS)
        return s.astype(np.float32), (bt - mu * s).astype(np.float32)

    s2, t2 = bn_affine('bn2')
    s3, t3 = bn_affine('bn3')
    s4, t4 = bn_affine('bn4')

    def col(v):
        return np.ascontiguousarray(np.asarray(v, np.float32).reshape(-1, 1))

    def pack_w(Wc, cin, cout, chunks):
        out = np.zeros((128, len(chunks), cout), np.float32)
        for c, (dy, dxe, kp) in enumerate(chunks):
            ngr = kp // cin
            for qi in range(ngr):
                out[cin * qi:cin * (qi + 1), c, :] = Wc[dy, qi + dxe]
        return out

    ch6 = [(d, 0, 48) for d in range(6)]
    W2sb = pack_w(n(p['cb2_w']), 20, 30, CH2)
    W3sb = pack_w(n(p['cb3_w']), 30, 40, CH3)
    W4sb = pack_w(n(p['cb4_w']), 40, 50, CH4)
    Wapc = np.concatenate([n(p['amp_w']), PI * n(p['phase_w'])], axis=3)
    Wapsb = pack_w(Wapc, 50, 8, CHAP)
    W6sb = pack_w(n(p['out_w']), 8, 4, ch6)[0:48]

    bf = lambda a: np.asarray(a).astype(ml_dtypes.bfloat16)
    shared = dict(
        lhs1=bf(lhs1),
        b1=col(n(p['cb1_b'])), s2=col(s2), t2=col(t2),
        W2sb=bf(W2sb), b2=col(n(p['cb2_b'])), s3=col(s3), t3=col(t3),
        W3sb=bf(W3sb), b3=col(n(p['cb3_b'])), s4=col(s4), t4=col(t4),
        W4sb=bf(W4sb), b4=col(n(p['cb4_b'])),
        Wap=bf(Wapsb),
        Wk=bf(n(p['k_w'])[0, 0]), Wv=bf(n(p['v_w'])[0, 0]), Wq=bf(n(p['q_w'])[0, 0]),
        Wl=bf(n(p['last_w'])[0, 0]), W6sb=bf(W6sb),
    )

    in_maps = []
    for core in range(8):
        b, r = core // 4, core % 4
        mask = np.ones((4736,), np.float32)
        if r == 0:
            mask[0:256] = 0
        if r == 3:
            mask[34 * 128:] = 0
        mask = np.broadcast_to(mask.reshape(1, 37, 128), (8, 37, 128)).copy()
        sel4 = np.zeros((32, 8), np.float32)
        for c in range(8):
            sel4[8 * r + c, c] = 1.0
        sel4 = sel4.astype(ml_dtypes.bfloat16)
        m = dict(shared)
        m['P'] = Pms[b].astype(ml_dtypes.bfloat16)
        m['mask'] = mask
        m['sel4'] = sel4
        in_maps.append(m)
    return in_maps


def host_assemble(results, sigma):
    out = np.zeros((2, 128, 128, 4), np.float32)
    for b in range(2):
        Z = np.float64(0)
        for r in range(4):
            Z += results[4 * b + r]['z'][0, 256:4352].sum(dtype=np.float64)
        Z = np.float32(Z)
        for r in range(4):
            res = results[4 * b + r]
            strip = res['A'][:, 256:4352] + (np.float32(sigma) / Z) * res['Bo'][:, 256:4352]
            out[b, 32 * r:32 * r + 32] = strip.reshape(4, 32, 128).transpose(1, 2, 0)
    return out


def kernel(x, params):
    x = np.asarray(x)
    if 'nc' not in _NC_CACHE:
        _NC_CACHE['nc'] = build_nc()
    nc = _NC_CACHE['nc']
    in_maps = host_prep(x, params)
    res = run_bass_kernel_spmd(nc, in_maps, core_ids=list(range(8)))
    return host_assemble(res.results, np.asarray(params['sigma']))


if __name__ == '__main__':
    nc = build_nc()
    print("built ok; instructions:",
          sum(len(bb.instructions) for fn in nc.m.functions for bb in fn.blocks))
